# revision 5
# baseline (speedup 1.0000x reference)
"""Conv2D 3x3 (stride 1, pad 1) NCHW kernel for Trainium2, 8 NeuronCores.

Reference op: y = conv2d(x[32,128,56,56], w[256,128,3,3]) + b[256]  (fp32)

Strategy:
  - Data-parallel over batch: 4 images per core, 8 cores.
  - Conv as shifted matmuls accumulating in PSUM over the 9 taps:
      out[co, (h,w)] = sum_{kh,kw} W[kh,kw][ci,co].T @ xpad[ci, h+kh, w+kw]
    K = ci = 128 sits exactly on the 128 SBUF partitions.
  - fp8e4m3 DoubleRow matmuls (0.5 PE cycles/row, 2x the fp16 rate), with a
    hi/lo split to keep precision: host pre-computes x_hi=Q8(x),
    x_lo=Q8(x-x_hi), w_hi=Q8(w), w_lo=Q8(w-w_hi). Each DoubleRow matmul
    contracts 2 k-tiles (dim1 of both APs). Per output tile (14 DR matmuls):
      * 9 "main":  lhsT=(w_hi[t], w_lo[t]),  rhs=(x_hi win, x_hi win)  [stride-0
        broadcast] => exact-w @ x_hi for every tap t
      * 4 "corr":  lhsT=(w_hi[A], w_hi[B]),  rhs=(x_lo winA, x_lo winB) for tap
        pairs (0,1),(2,3),(4,5),(6,7) => w_hi @ x_lo
      * 1 "corr8": lhsT=(w_hi[8], w_lo[8]),  rhs=(x_lo win8 broadcast)
        => exact-w @ x_lo for tap 8
    Dropped term: sum_{t<8} w_lo[t] @ x_lo[t]  ~= 1e-3 relative. PSUM fp32.
    3136 PE cycles/tile vs 4032 for fp16 (9 matmuls @ 448 cycles).
  - Host pre-pads x to 58x58 (zero halo); weight pre-transposed to
    [ci, tap, {hi,lo}, co]; the device never transposes or memsets.
  - Output tiles are 8 rows x 56 cols = 448 = one PSUM bank; bias is fused
    into the PSUM->SBUF eviction (Identity+bias) which emits fp16; the host
    upcasts to fp32 after gathering (halves the store traffic).
"""

import numpy as np
import ml_dtypes

N_CORES = 8
B, CI, H, W = 32, 128, 56, 56
CO = 256
KH = KW = 3
NTAPS = KH * KW
BS = B // N_CORES            # images per core
HP, WP = H + 2, W + 2        # padded image
HB = 8                       # output rows per block
NB = H // HB                 # blocks per image
NTILE = HB * W               # 448 fp32 -> one PSUM bank
NCHUNK = CO // 128           # co chunks of 128 partitions

F8 = ml_dtypes.float8_e4m3   # maps to mybir.dt.float8e4
# The reference weights are xavier-uniform with bound 0.042 — right in fp8
# e4m3's denormal range (min normal 2^-6), where quantization error is a
# fixed 2^-10 step and a hi/lo split cannot refine it. Pre-scale weights by
# 2^6 into the well-resolved range; the PSUM eviction divides back via the
# activation's scale operand (out = psum * 1/WSCALE + bias).
WSCALE = 64.0

# tap pairs for the x_lo correction, and the SBUF element offset between
# the two windows: tap t=(kh,kw) -> window start kh*WP + kw
_CPAIRS = [(0, 1), (2, 3), (4, 5), (6, 7)]
_TAPOFF = [(t // KW) * WP + (t % KW) for t in range(NTAPS)]

_cache = {}


def _build(reps=1, internal_io=False):
    """Build + compile the SPMD program.

    reps>1 wraps the whole load+compute+store body in a For_i hardware
    loop (used for timing). internal_io keeps x/y in device DRAM with a
    small token output (timing-only: no host transfer of the big tensors).
    """
    import contextlib

    import concourse.mybir as mybir
    import concourse.tile as tile
    from concourse import bacc
    from concourse.ap import AP

    f8 = mybir.dt.float8e4
    nc = bacc.Bacc("TRN2", target_bir_lowering=False, debug=False)
    if internal_io:
        xq_ap = nc.dram_tensor("xq_i", [2, BS, CI, HP, WP], f8).ap()
        y_ap = nc.dram_tensor("y_i", [BS, CO, H, W], mybir.dt.float16).ap()
        tok_ap = nc.dram_tensor(
            "tok", [128, NCHUNK], mybir.dt.float32, kind="ExternalOutput"
        ).ap()
    else:
        xq_ap = nc.dram_tensor(
            "xq", [2, BS, CI, HP, WP], f8, kind="ExternalInput"
        ).ap()
        y_ap = nc.dram_tensor(
            "y", [BS, CO, H, W], mybir.dt.float16, kind="ExternalOutput"
        ).ap()
        tok_ap = None
    wt_ap = nc.dram_tensor(
        "wt", [CI, NTAPS * 2 * CO], f8, kind="ExternalInput"
    ).ap()
    bt_ap = nc.dram_tensor(
        "bt", [128, NCHUNK], mybir.dt.float32, kind="ExternalInput"
    ).ap()

    IMSZ = HP * WP               # elements per (s, img) per partition

    with tile.TileContext(nc) as tc:
        with (
            tc.tile_pool(name="xw", bufs=1) as xw,
            tc.tile_pool(name="out", bufs=8) as outp,
            tc.tile_pool(name="ps", bufs=7, space="PSUM") as ps,
        ):
            wsb = xw.tile([CI, NTAPS * 2 * CO], f8, tag="w")
            bsb = xw.tile([128, NCHUNK], mybir.dt.float32, tag="b")
            # weights on the scalar engine's HWDGE queues: parallel with the
            # x loads below (gpsimd SWDGE queue)
            nc.scalar.dma_start(out=wsb[:], in_=wt_ap[:, :])
            nc.scalar.dma_start(out=bsb[:], in_=bt_ap[:, :])

            loop_cm = (
                tc.For_i(0, reps, 1, hint_engines=(mybir.EngineType.PE,))
                if reps > 1
                else contextlib.nullcontext()
            )
            with loop_cm:
                # [128, {hi,lo} x img x 58 x 58]
                xsb = xw.tile([CI, 2 * BS * IMSZ], f8, tag="x")
                xdma = nc.gpsimd
                # priority chunks: rows 0..HB+1 of img0 hi AND lo --
                # everything the first tile's matmuls read -- so PE starts
                # ~1us in
                head = (HB + 2) * WP
                for s in range(2):
                    xflat0 = xq_ap[s, 0].rearrange("c h w -> c (h w)")
                    o = s * BS * IMSZ
                    xdma.dma_start(
                        out=xsb[:, o : o + head], in_=xflat0[:, 0:head]
                    )
                for s in range(2):
                    xflat0 = xq_ap[s, 0].rearrange("c h w -> c (h w)")
                    o = s * BS * IMSZ
                    xdma.dma_start(
                        out=xsb[:, o + head : o + IMSZ],
                        in_=xflat0[:, head:IMSZ],
                    )
                for img in range(1, BS):
                    for s in range(2):
                        o = s * BS * IMSZ + img * IMSZ
                        xdma.dma_start(
                            out=xsb[:, o : o + IMSZ],
                            in_=xq_ap[s, img].rearrange("c h w -> c (h w)")[
                                :, :
                            ],
                        )
                # [c, s, img, h, w] view of SBUF x
                xv = xsb[:].rearrange(
                    "c (s n h w) -> c s n h w", s=2, n=BS, h=HP
                )
                # [c, tap, {hi,lo}, co] view of weights
                wv = wsb[:].rearrange("c (t p m) -> c t p m", t=NTAPS, p=2)

                DR = mybir.MatmulPerfMode.DoubleRow

                def win(s, img, r0, kw):
                    return xv[:, s, img, r0 : r0 + HB, kw : kw + W]

                for c in range(NCHUNK):
                    cs = slice(c * 128, (c + 1) * 128)
                    for img in range(BS):
                        for hb in range(NB):
                            pt = ps.tile(
                                [128, NTILE], mybir.dt.float32, tag="acc"
                            )
                            # 9 main: exact-w @ x_hi
                            for t in range(NTAPS):
                                kh, kw = t // KW, t % KW
                                rhs = (
                                    win(0, img, hb * HB + kh, kw)
                                    .unsqueeze(1)
                                    .broadcast_to([128, 2, HB, W])
                                )
                                nc.tensor.matmul(
                                    pt[:],
                                    wv[:, t, :, cs],
                                    rhs,
                                    start=(t == 0),
                                    stop=False,
                                    perf_mode=DR,
                                )
                            # 4 corr: w_hi @ x_lo for tap pairs
                            for ta, tb in _CPAIRS:
                                kh, kw = ta // KW, ta % KW
                                wa = win(1, img, hb * HB + kh, kw)
                                delta = _TAPOFF[tb] - _TAPOFF[ta]
                                rhs = AP(
                                    wa.tensor,
                                    wa.offset,
                                    [list(wa.ap[0]), [delta, 2]]
                                    + [list(d) for d in wa.ap[1:]],
                                )
                                nc.tensor.matmul(
                                    pt[:],
                                    wv[:, ta : tb + 1, 0, cs],
                                    rhs,
                                    start=False,
                                    stop=False,
                                    perf_mode=DR,
                                )
                            # 1 corr8: exact-w @ x_lo for tap 8
                            rhs = (
                                win(1, img, hb * HB + 2, 2)
                                .unsqueeze(1)
                                .broadcast_to([128, 2, HB, W])
                            )
                            nc.tensor.matmul(
                                pt[:],
                                wv[:, 8, :, cs],
                                rhs,
                                start=False,
                                stop=True,
                                perf_mode=DR,
                            )
                            ot = outp.tile(
                                [128, NTILE], mybir.dt.float16, tag="o"
                            )
                            nc.scalar.activation(
                                ot[:],
                                pt[:],
                                mybir.ActivationFunctionType.Identity,
                                bias=bsb[:, c : c + 1],
                                scale=1.0 / WSCALE,
                            )
                            nc.sync.dma_start(
                                out=y_ap[
                                    img,
                                    c * 128 : (c + 1) * 128,
                                    hb * HB : (hb + 1) * HB,
                                    :,
                                ],
                                in_=ot[:],
                            )
            if tok_ap is not None:
                nc.sync.dma_start(out=tok_ap[:, :], in_=bsb[:])
    nc.compile()
    return nc


def _get_nc(reps=1, wdtype=None, internal_io=False):
    key = (reps, internal_io)
    if key not in _cache:
        _cache[key] = _build(reps, internal_io)
    return _cache[key]


def _prep_inputs(x, weight, bias):
    x = np.asarray(x, dtype=np.float32)
    weight = np.ascontiguousarray(weight, dtype=np.float32)
    bias = np.ascontiguousarray(bias, dtype=np.float32)
    # fused pad + hi/lo fp8 quantization: zero halo quantizes to zero
    xpad = np.zeros((B, CI, HP, WP), dtype=np.float32)
    xpad[:, :, 1 : H + 1, 1 : W + 1] = x
    x_hi = xpad.astype(F8)
    x_lo = (xpad - x_hi.astype(np.float32)).astype(F8)
    # [co, ci, kh, kw] -> [ci, tap, {hi,lo}, co] flattened, pre-scaled
    wt32 = weight.transpose(1, 2, 3, 0).reshape(CI, NTAPS, 1, CO) * WSCALE
    w_hi = wt32.astype(F8)
    w_lo = (wt32 - w_hi.astype(np.float32)).astype(F8)
    wt = np.ascontiguousarray(
        np.concatenate([w_hi, w_lo], axis=2).reshape(CI, NTAPS * 2 * CO)
    )
    bt = np.ascontiguousarray(bias.reshape(NCHUNK, 128).T)
    in_maps = [
        {
            "xq": np.ascontiguousarray(
                np.stack(
                    [x_hi[i * BS : (i + 1) * BS], x_lo[i * BS : (i + 1) * BS]]
                )
            ),
            "wt": wt,
            "bt": bt,
        }
        for i in range(N_CORES)
    ]
    return in_maps


def run_sharded(x, weight, bias, trace=False, reps=1):
    """Run on all 8 cores; returns (full_output, BassKernelResults)."""
    from concourse.bass_utils import run_bass_kernel_spmd

    nc = _get_nc(reps)
    in_maps = _prep_inputs(x, weight, bias)
    res = run_bass_kernel_spmd(nc, in_maps, list(range(N_CORES)), trace=trace)
    y = np.concatenate(
        [res.results[i]["y"].astype(np.float32) for i in range(N_CORES)],
        axis=0,
    )
    return y, res


def kernel(x, weight, bias):
    y, _ = run_sharded(x, weight, bias)
    return y


# revision 6
# speedup vs baseline: 1.4915x; 1.4915x over previous
"""Conv2D 3x3 (stride 1, pad 1) NCHW kernel for Trainium2, 8 NeuronCores.

Reference op: y = conv2d(x[32,128,56,56], w[256,128,3,3]) + b[256]  (fp32)

Strategy:
  - Data-parallel over batch: 4 images per core, 8 cores.
  - Conv as 9 shifted matmuls accumulating in PSUM:
      out[co, (h,w)] = sum_{kh,kw} W[kh,kw][ci,co].T @ xpad[ci, h+kh, w+kw]
    K = ci = 128 sits exactly on the 128 SBUF partitions.
  - Host pre-pads x to 58x58 (zero halo) and pre-transposes the weight to
    [ci, (kh kw), co], so the device never transposes or memsets anything.
  - fp16 operands (host-cast): full PE rate (1 col/cycle); the 128-col
    LDWEIGHTS rides FWL + the PE reorder window and hides under the
    448-cycle matmuls (measured: 94.6us steady state = 100.5% of the
    504-matmul PE roofline).
    [fp8 DoubleRow was tried and measured SLOWER (153us): DoubleRow
     disables FWL and its 256-col weight loads serialize with the 224-cycle
     matmuls. See kernel_fp8_dr.py.]
  - Output tiles are 8 rows x 56 cols = 448 fp32 = one PSUM bank; bias is
    fused into the PSUM->SBUF eviction on the Scalar engine (Identity+bias)
    which emits fp16 (halves eviction writes + store DMA); host upcasts to
    fp32 after the gather. Measured full-conv rel err ~= 3e-4.
"""

import numpy as np

N_CORES = 8
B, CI, H, W = 32, 128, 56, 56
CO = 256
KH = KW = 3
NTAPS = KH * KW
BS = B // N_CORES            # images per core
HP, WP = H + 2, W + 2        # padded image
HB = 8                       # output rows per block
NB = H // HB                 # blocks per image
NTILE = HB * W               # 448 fp32 -> one PSUM bank
NCHUNK = CO // 128           # co chunks of 128 partitions

WDTYPE = "float16"

_cache = {}


def _build(reps=1, internal_io=False):
    """Build + compile the SPMD program.

    reps>1 wraps the whole load+compute+store body in a For_i hardware
    loop (used for timing). internal_io keeps x/y in device DRAM with a
    small token output (timing-only: no host transfer of the big tensors).
    """
    import contextlib

    import concourse.mybir as mybir
    import concourse.tile as tile
    from concourse import bacc

    mmdt = mybir.dt.float16

    nc = bacc.Bacc("TRN2", target_bir_lowering=False, debug=False)
    if internal_io:
        xp_ap = nc.dram_tensor("xp_i", [BS, CI, HP, WP], mmdt).ap()
        y_ap = nc.dram_tensor("y_i", [BS, CO, H, W], mybir.dt.float16).ap()
        tok_ap = nc.dram_tensor(
            "tok", [128, NCHUNK], mybir.dt.float32, kind="ExternalOutput"
        ).ap()
    else:
        xp_ap = nc.dram_tensor(
            "xp", [BS, CI, HP, WP], mmdt, kind="ExternalInput"
        ).ap()
        y_ap = nc.dram_tensor(
            "y", [BS, CO, H, W], mybir.dt.float16, kind="ExternalOutput"
        ).ap()
        tok_ap = None
    wt_ap = nc.dram_tensor(
        "wt", [CI, NTAPS * CO], mmdt, kind="ExternalInput"
    ).ap()
    bt_ap = nc.dram_tensor(
        "bt", [128, NCHUNK], mybir.dt.float32, kind="ExternalInput"
    ).ap()

    with tile.TileContext(nc) as tc:
        with (
            tc.tile_pool(name="xw", bufs=1) as xw,
            tc.tile_pool(name="out", bufs=8) as outp,
            tc.tile_pool(name="ps", bufs=7, space="PSUM") as ps,
        ):
            wsb = xw.tile([CI, NTAPS * CO], mmdt, tag="w")
            bsb = xw.tile([128, NCHUNK], mybir.dt.float32, tag="b")
            # weights on the scalar engine's HWDGE queues: parallel
            # with the x loads below (gpsimd SWDGE queue)
            nc.scalar.dma_start(out=wsb[:], in_=wt_ap[:, :])
            nc.scalar.dma_start(out=bsb[:], in_=bt_ap[:, :])

            loop_cm = (
                tc.For_i(0, reps, 1, hint_engines=(mybir.EngineType.PE,))
                if reps > 1
                else contextlib.nullcontext()
            )
            with loop_cm:
                xsb = xw.tile([CI, BS * HP * WP], mmdt, tag="x")
                xdma = nc.gpsimd
                # priority chunk: rows 0..HB+1 of img0 — everything the first
                # matmul group reads — so PE can start ~1us in
                head = (HB + 2) * WP
                xflat0 = xp_ap[0].rearrange("c h w -> c (h w)")
                xdma.dma_start(out=xsb[:, 0:head], in_=xflat0[:, 0:head])
                xdma.dma_start(
                    out=xsb[:, head : HP * WP], in_=xflat0[:, head : HP * WP]
                )
                for img in range(1, BS):
                    xdma.dma_start(
                        out=xsb[:, img * HP * WP : (img + 1) * HP * WP],
                        in_=xp_ap[img].rearrange("c h w -> c (h w)")[:, :],
                    )
                xv = xsb[:].rearrange("c (n h w) -> c n h w", n=BS, h=HP)

                for c in range(NCHUNK):
                    for img in range(BS):
                        for hb in range(NB):
                            pt = ps.tile([128, NTILE], mybir.dt.float32, tag="acc")
                            for kh in range(KH):
                                for kw in range(KW):
                                    tap = kh * KW + kw
                                    r0 = hb * HB + kh
                                    nc.tensor.matmul(
                                        pt[:],
                                        wsb[
                                            :,
                                            tap * CO
                                            + c * 128 : tap * CO
                                            + (c + 1) * 128,
                                        ],
                                        xv[:, img, r0 : r0 + HB, kw : kw + W],
                                        start=(tap == 0),
                                        stop=(tap == NTAPS - 1),
                                    )
                            ot = outp.tile([128, NTILE], mybir.dt.float16, tag="o")
                            nc.scalar.activation(
                                ot[:],
                                pt[:],
                                mybir.ActivationFunctionType.Identity,
                                bias=bsb[:, c : c + 1],
                                scale=1.0,
                            )
                            nc.sync.dma_start(
                                out=y_ap[
                                    img,
                                    c * 128 : (c + 1) * 128,
                                    hb * HB : (hb + 1) * HB,
                                    :,
                                ],
                                in_=ot[:],
                            )
            if tok_ap is not None:
                nc.sync.dma_start(out=tok_ap[:, :], in_=bsb[:])
    nc.compile()
    return nc


def _get_nc(reps=1, wdtype=None, internal_io=False):
    key = (reps, internal_io)
    if key not in _cache:
        _cache[key] = _build(reps, internal_io)
    return _cache[key]


def _prep_inputs(x, weight, bias):
    npdt = np.float16
    x = np.asarray(x)
    weight = np.ascontiguousarray(weight, dtype=np.float32)
    bias = np.ascontiguousarray(bias, dtype=np.float32)
    # fused pad+cast: one pass over x instead of pad(fp32) then astype
    xpad = np.zeros((B, CI, HP, WP), dtype=npdt)
    xpad[:, :, 1 : H + 1, 1 : W + 1] = x
    # [co, ci, kh, kw] -> [ci, (kh kw), co] flattened to [ci, 9*co]
    wt = np.ascontiguousarray(
        weight.transpose(1, 2, 3, 0).reshape(CI, NTAPS * CO).astype(npdt)
    )
    bt = np.ascontiguousarray(bias.reshape(NCHUNK, 128).T)
    in_maps = [
        {
            "xp": np.ascontiguousarray(xpad[i * BS : (i + 1) * BS]),
            "wt": wt,
            "bt": bt,
        }
        for i in range(N_CORES)
    ]
    return in_maps


def run_sharded(x, weight, bias, trace=False, reps=1):
    """Run on all 8 cores; returns (full_output, BassKernelResults)."""
    from concourse.bass_utils import run_bass_kernel_spmd

    nc = _get_nc(reps)
    in_maps = _prep_inputs(x, weight, bias)
    res = run_bass_kernel_spmd(nc, in_maps, list(range(N_CORES)), trace=trace)
    y = np.concatenate(
        [res.results[i]["y"].astype(np.float32) for i in range(N_CORES)],
        axis=0,
    )
    return y, res


def kernel(x, weight, bias):
    y, _ = run_sharded(x, weight, bias)
    return y


# revision 10
# speedup vs baseline: 1.5130x; 1.0144x over previous
"""Conv2D 3x3 (stride 1, pad 1) NCHW kernel for Trainium2, 8 NeuronCores.

Reference op: y = conv2d(x[32,128,56,56], w[256,128,3,3]) + b[256]  (fp32)

Strategy:
  - Data-parallel over batch: 4 images per core, 8 cores.
  - Conv as 9 shifted matmuls accumulating in PSUM:
      out[co, (h,w)] = sum_{kh,kw} W[kh,kw][ci,co].T @ xpad[ci, h+kh, w+kw]
    K = ci = 128 sits exactly on the 128 SBUF partitions.
  - Host pre-pads x to 58x58 (zero halo) and pre-transposes the weight to
    [ci, (kh kw), co], so the device never transposes or memsets anything.
  - fp16 operands (host-cast): full PE rate (1 col/cycle); the 128-col
    LDWEIGHTS rides FWL + the PE reorder window and hides under the
    448-cycle matmuls (measured: 94.6us steady state = 100.5% of the
    504-matmul PE roofline).
    [fp8 DoubleRow was tried and measured SLOWER (153us): DoubleRow
     disables FWL and its 256-col weight loads serialize with the 224-cycle
     matmuls. See kernel_fp8_dr.py.]
  - Output tiles are 8 rows x 56 cols = 448 fp32 = one PSUM bank; bias is
    fused into the PSUM->SBUF eviction on the Scalar engine (Identity+bias)
    which emits fp16 (halves eviction writes + store DMA); host upcasts to
    fp32 after the gather. Measured full-conv rel err ~= 3e-4.
"""

import numpy as np

N_CORES = 8
B, CI, H, W = 32, 128, 56, 56
CO = 256
KH = KW = 3
NTAPS = KH * KW
BS = B // N_CORES            # images per core
HP, WP = H + 2, W + 2        # padded image
HB = 8                       # output rows per block
NB = H // HB                 # blocks per image
NTILE = HB * W               # 448 fp32 -> one PSUM bank
NCHUNK = CO // 128           # co chunks of 128 partitions

WDTYPE = "float16"

_cache = {}


def _build(reps=1, internal_io=False):
    """Build + compile the SPMD program.

    reps>1 wraps the whole load+compute+store body in a For_i hardware
    loop (used for timing). internal_io keeps x/y in device DRAM with a
    small token output (timing-only: no host transfer of the big tensors).
    """
    import contextlib

    import concourse.mybir as mybir
    import concourse.tile as tile
    from concourse import bacc

    mmdt = mybir.dt.float16

    nc = bacc.Bacc("TRN2", target_bir_lowering=False, debug=False)
    if internal_io:
        xp_ap = nc.dram_tensor("xp_i", [BS, CI, HP, WP], mmdt).ap()
        y_ap = nc.dram_tensor("y_i", [BS, CO, H, W], mybir.dt.float16).ap()
        tok_ap = nc.dram_tensor(
            "tok", [128, NCHUNK], mybir.dt.float32, kind="ExternalOutput"
        ).ap()
    else:
        xp_ap = nc.dram_tensor(
            "xp", [BS, CI, HP, WP], mmdt, kind="ExternalInput"
        ).ap()
        y_ap = nc.dram_tensor(
            "y", [BS, CO, H, W], mybir.dt.float16, kind="ExternalOutput"
        ).ap()
        tok_ap = None
    wt_ap = nc.dram_tensor(
        "wt", [CI, NTAPS * CO], mmdt, kind="ExternalInput"
    ).ap()
    bt_ap = nc.dram_tensor(
        "bt", [128, NCHUNK], mybir.dt.float32, kind="ExternalInput"
    ).ap()

    with tile.TileContext(nc) as tc:
        with (
            tc.tile_pool(name="xw", bufs=1) as xw,
            tc.tile_pool(name="out", bufs=8) as outp,
            tc.tile_pool(name="ps", bufs=7, space="PSUM") as ps,
        ):
            wsb = xw.tile([CI, NTAPS * CO], mmdt, tag="w")
            bsb = xw.tile([128, NCHUNK], mybir.dt.float32, tag="b")
            # weights on the scalar engine's HWDGE queues: parallel
            # with the x loads below (gpsimd SWDGE queue)
            nc.scalar.dma_start(out=wsb[:], in_=wt_ap[:, :])
            nc.scalar.dma_start(out=bsb[:], in_=bt_ap[:, :])

            # PE p-state warmup: the tensor engine runs at half clock until
            # it has been busy ~3-4us (HAM K-state ramp). These matmuls have
            # no data dependencies (never-written SBUF, dead PSUM tile) so
            # they issue at t=0 and ramp the PE while the weight/x DMAs are
            # still in flight; the real matmuls then start at full clock.
            wuw = xw.tile([128, 128], mmdt, tag="wuw")
            wux = xw.tile([128, NTILE], mmdt, tag="wux")
            wup = ps.tile([128, NTILE], mybir.dt.float32, tag="wup", bufs=1)
            nc.vector.memset(wuw[:], 0.0)
            nc.vector.memset(wux[:], 0.0)
            for _ in range(10):
                nc.tensor.matmul(
                    wup[:], wuw[:], wux[:], start=True, stop=True,
                    skip_group_check=True,
                )

            loop_cm = (
                tc.For_i(0, reps, 1, hint_engines=(mybir.EngineType.PE,))
                if reps > 1
                else contextlib.nullcontext()
            )
            with loop_cm:
                xsb = xw.tile([CI, BS * HP * WP], mmdt, tag="x")
                xdma = nc.gpsimd
                # priority chunk: rows 0..HB+1 of img0 — everything the first
                # matmul group reads — so PE can start ~1us in
                head = (HB + 2) * WP
                xflat0 = xp_ap[0].rearrange("c h w -> c (h w)")
                xdma.dma_start(out=xsb[:, 0:head], in_=xflat0[:, 0:head])
                xdma.dma_start(
                    out=xsb[:, head : HP * WP], in_=xflat0[:, head : HP * WP]
                )
                for img in range(1, BS):
                    xdma.dma_start(
                        out=xsb[:, img * HP * WP : (img + 1) * HP * WP],
                        in_=xp_ap[img].rearrange("c h w -> c (h w)")[:, :],
                    )
                xv = xsb[:].rearrange("c (n h w) -> c n h w", n=BS, h=HP)

                for c in range(NCHUNK):
                    for img in range(BS):
                        for hb in range(NB):
                            pt = ps.tile([128, NTILE], mybir.dt.float32, tag="acc")
                            for kh in range(KH):
                                for kw in range(KW):
                                    tap = kh * KW + kw
                                    r0 = hb * HB + kh
                                    nc.tensor.matmul(
                                        pt[:],
                                        wsb[
                                            :,
                                            tap * CO
                                            + c * 128 : tap * CO
                                            + (c + 1) * 128,
                                        ],
                                        xv[:, img, r0 : r0 + HB, kw : kw + W],
                                        start=(tap == 0),
                                        stop=(tap == NTAPS - 1),
                                    )
                            ot = outp.tile([128, NTILE], mybir.dt.float16, tag="o")
                            nc.scalar.activation(
                                ot[:],
                                pt[:],
                                mybir.ActivationFunctionType.Identity,
                                bias=bsb[:, c : c + 1],
                                scale=1.0,
                            )
                            nc.sync.dma_start(
                                out=y_ap[
                                    img,
                                    c * 128 : (c + 1) * 128,
                                    hb * HB : (hb + 1) * HB,
                                    :,
                                ],
                                in_=ot[:],
                            )
            if tok_ap is not None:
                nc.sync.dma_start(out=tok_ap[:, :], in_=bsb[:])
    nc.compile()
    return nc


def _get_nc(reps=1, wdtype=None, internal_io=False):
    key = (reps, internal_io)
    if key not in _cache:
        _cache[key] = _build(reps, internal_io)
    return _cache[key]


def _prep_inputs(x, weight, bias):
    npdt = np.float16
    x = np.asarray(x)
    weight = np.ascontiguousarray(weight, dtype=np.float32)
    bias = np.ascontiguousarray(bias, dtype=np.float32)
    # fused pad+cast: one pass over x instead of pad(fp32) then astype
    xpad = np.zeros((B, CI, HP, WP), dtype=npdt)
    xpad[:, :, 1 : H + 1, 1 : W + 1] = x
    # [co, ci, kh, kw] -> [ci, (kh kw), co] flattened to [ci, 9*co]
    wt = np.ascontiguousarray(
        weight.transpose(1, 2, 3, 0).reshape(CI, NTAPS * CO).astype(npdt)
    )
    bt = np.ascontiguousarray(bias.reshape(NCHUNK, 128).T)
    in_maps = [
        {
            "xp": np.ascontiguousarray(xpad[i * BS : (i + 1) * BS]),
            "wt": wt,
            "bt": bt,
        }
        for i in range(N_CORES)
    ]
    return in_maps


def run_sharded(x, weight, bias, trace=False, reps=1):
    """Run on all 8 cores; returns (full_output, BassKernelResults)."""
    from concourse.bass_utils import run_bass_kernel_spmd

    nc = _get_nc(reps)
    in_maps = _prep_inputs(x, weight, bias)
    res = run_bass_kernel_spmd(nc, in_maps, list(range(N_CORES)), trace=trace)
    y = np.concatenate(
        [res.results[i]["y"].astype(np.float32) for i in range(N_CORES)],
        axis=0,
    )
    return y, res


def kernel(x, weight, bias):
    y, _ = run_sharded(x, weight, bias)
    return y
